# revision 10
# baseline (speedup 1.0000x reference)
# Trainium2 Bass kernel for AttentionBlock (conv-qkv + spatial softmax attention
# + 1x1 conv out + residual), data-parallel over batch on 8 NeuronCores.
#
# v2: engine-balanced rewrite of the fp16 baseline.
#   - All convs run from an img2col fp8 image (xc [c, 9tap, 4096pix], built on
#     host) as DoubleRow fp8 matmuls (2 taps contracted per pass, 0.5 cyc/row).
#   - The u-conv (Wo.Wv folded) is computed directly TRANSPOSED on the PE
#     (stationary = image window, moving = weights) so uT needs no transposes.
#   - Attention: S = k^T q stays fp16 (PSUM-write-bound either way); exp runs
#     on ACT with fp8e4 output; O = uT^T E runs as DoubleRow fp8 over jt-pairs.
#   - colsum (softmax denominator) accumulates in fp16 via DVE+Pool split.
#   - Conv work is streamed into the attention loop (ACT is the critical
#     engine; PE slack absorbs the convs), with explicit per-slot work lists.
#   - Weights are scaled x32 on host to dodge fp8 subnormals; the 1/32 (and
#     the 1/1024 logit scale) fold into the activation scale and the final
#     normalize (scalar_tensor_tensor), costing nothing.
#   - bias bu = Wo bv + bo commutes with the row-stochastic A and is folded
#     into the residual (host adds it to xres). bk cancels in softmax but is
#     applied anyway; bq is applied on q eviction.

import numpy as np

try:
    import concourse.bass as bass  # noqa: F401
except ImportError:  # pragma: no cover
    import sys

    sys.path.insert(0, "/opt/trn_rl_repo")

import concourse.bass as bass  # noqa: F401
import concourse.mybir as mybir
import ml_dtypes
from concourse import bacc
from concourse import tile

B = 8
C = 128
H = W = 64
N = H * W  # 4096
NTAP = 9
IB = 1024  # attention i-block (columns per PSUM residency)
NIB = N // IB  # 4
NJT = N // 128  # 32 j-tiles per ib
NSLOT = NIB * NJT  # 128 global slots
WSCALE = 32.0  # fp8 weight pre-scale (host)
SCALE = float(C) ** -0.5
EXP_BIAS = -3.0

F32 = mybir.dt.float32
F16 = mybir.dt.float16
F8 = mybir.dt.float8e4
NP8 = ml_dtypes.float8_e4m3
DRm = mybir.MatmulPerfMode.DoubleRow

# DVE-exp share (ib>=1): the last ED columns of each S tile are exponentiated
# on the DVE via the int16 Schraudolph bit-trick (bits of fp16 2^x built by a
# single mult+add+convert), written straight into e2's fp16 tail columns so
# the O matmul and the colsum read one uniform fp16 tile. The 2^K0 offset the
# trick introduces is uniform per column and cancels in the per-column
# softmax normalization. K0 is chosen so the int16 can never go negative
# (negative bit patterns would alias to fp16 -NaN).
ED = 192  # 0 disables the DVE-exp path
AE = IB - ED  # ACT-exp columns per tile in ib>=1
A16 = 1024.0 * 1.4426950408889634
K0 = 17156.0

_CACHE = {}


def _build_nc():
    nc = bacc.Bacc(None)

    xc_d = nc.dram_tensor("xc", [C, NTAP, N], F8, kind="ExternalInput")
    # weights and biases packed into single tensors: DMA *issue* time on the
    # sequencer (~0.6-1.3us each) dominates the prologue, so fewer DMAs win.
    wks_d = nc.dram_tensor("wks", [C, 3, NTAP, C], F8, kind="ExternalInput")
    bqk_d = nc.dram_tensor("bqk", [C, 2], F32, kind="ExternalInput")
    xr_d = nc.dram_tensor("xr", [C, N], F16, kind="ExternalInput")
    out_d = nc.dram_tensor("out", [C, H, W], F16, kind="ExternalOutput")

    with tile.TileContext(nc) as tc:
        with tc.tile_pool(name="persist", bufs=1) as pp:
            xc = pp.tile([C, NTAP, N], F8)
            xres = pp.tile([C, N], F16)
            qb = pp.tile([C, N], F16)
            kb = pp.tile([C, N], F16)
            uT = pp.tile([C, NJT, C], F16)  # [j-in-tile, jt, c]
            wks = pp.tile([C, 3, NTAP, C], F8)  # packed wk, wq, wu
            bqk = pp.tile([C, 2], F32)
            wk_s, wq_s, wu_s = wks[:, 0], wks[:, 1], wks[:, 2]
            bq_s, bk_s = bqk[:, 0:1], bqk[:, 1:2]
            ebias = pp.tile([C, 1], F32)
            ones = pp.tile([C, 1], F16)
            rcp = pp.tile([C, NIB, 8], F32)

            # -------- input DMAs, spread over 4 queues so the early chunks
            # land in parallel (per-queue issue+transfer is serial) --------
            # One queue, strictly in consumption order: both the issue slots
            # and the transfer channel are serial, so order is everything and
            # fewer DMAs are better.
            nc.sync.dma_start(wks, wks_d[:])
            nc.sync.dma_start(bqk, bqk_d[:])
            for ch in range(4):
                nc.sync.dma_start(
                    xc[:, :, ch * 1024 : (ch + 1) * 1024],
                    xc_d[:, :, ch * 1024 : (ch + 1) * 1024],
                )
            nc.sync.dma_start(xres, xr_d[:])
            # PE warmup: the tensor engine ramps to full clock only after
            # ~3us from its first instruction; issue throwaway matmuls on
            # memset data immediately so the ramp overlaps the input DMAs.
            wrm = pp.tile([C, 512], F16)
            nc.vector.memset(wrm, 0.0)
            nc.vector.memset(ebias, EXP_BIAS)
            # ones = WSCALE: folds the 1/WSCALE u-descale into the softmax
            # denominator (colsum*WSCALE), so normalize is a plain multiply.
            nc.vector.memset(ones, WSCALE)

            cps = tc.alloc_tile_pool(name="cps", bufs=2, space="PSUM")
            # continuous warmup until the first conv's inputs arrive — if the
            # PE goes idle the clock governor drops back to the mid p-state.
            wps = cps.tile([C, 512], F32, tag="conv", name="wps")
            for _ in range(8):
                nc.tensor.matmul(
                    wps[0:64, :], wrm[:, 0:64], wrm, start=True, stop=True
                )
            sps = tc.alloc_tile_pool(name="sps", bufs=2, space="PSUM")
            ops = tc.alloc_tile_pool(name="ops", bufs=1, space="PSUM")
            ep = tc.alloc_tile_pool(name="ep", bufs=4)
            ap = tc.alloc_tile_pool(name="accp", bufs=2)
            fin = tc.alloc_tile_pool(name="fin", bufs=2)
            dsp = tc.alloc_tile_pool(name="dstage", bufs=1, space="DRAM")
            rstage = dsp.tile([N], F32)

            # ---------------- conv emission helpers ----------------
            # q/k conv tile t covers pixels [t*512, (t+1)*512), normal
            # orientation (out channels on partitions). Emitted as 4 groups:
            # group 0 = tap 8 (plain fp8, the sim's group anchor) + pair 0,
            # groups 1,2 = pairs 1,2, group 3 = pair 3 + eviction (DVE).
            # The eviction is a full slot ahead of the next tile's group 0,
            # so its DVE round-trip hides under the S/O/u work in between.
            qk_ps = {}

            def qk_pair(ps, w_s, p0, pr):
                for ph in range(2):
                    nc.tensor.matmul(
                        ps[:, ph * 256 : (ph + 1) * 256],
                        w_s[:, 2 * pr : 2 * pr + 2, :],
                        xc[:, 2 * pr : 2 * pr + 2, p0 + ph * 256 : p0 + (ph + 1) * 256],
                        start=False, stop=(pr == 3 and ph == 1), perf_mode=DRm,
                    )

            def qk_group(w_s, b_s, dest, t, g):
                p0 = t * 512
                if g == 0:
                    ps = cps.tile([C, 512], F32, tag="conv", name="cps")
                    qk_ps[id(w_s), t] = ps
                    nc.tensor.matmul(
                        ps, w_s[:, 8, :], xc[:, 8, p0 : p0 + 512],
                        start=True, stop=False,
                    )
                    qk_pair(ps, w_s, p0, 0)
                    return
                ps = qk_ps[id(w_s), t]
                qk_pair(ps, w_s, p0, g)
                if g == 3:
                    del qk_ps[id(w_s), t]
                    nc.vector.tensor_scalar_add(
                        dest[:, p0 : p0 + 512], ps, b_s
                    )

            # u-conv j-tile j (transposed): stationary = image window,
            # moving = wu. Lands [128 pix, 128 ch] into a 4-jt psu batch.
            u_ps = {}

            def u_jt(j):
                b = j // 4
                if j % 4 == 0:
                    u_ps[b] = cps.tile([C, 512], F32, tag="conv", name="ups")
                ps = u_ps[b]
                sl = slice((j % 4) * 128, (j % 4 + 1) * 128)
                jp = j * 128
                first = j % 4 == 0
                last = j % 4 == 3
                nc.tensor.matmul(
                    ps[:, sl], xc[:, 8, jp : jp + 128], wu_s[:, 8, :],
                    start=first, stop=False,
                )
                for pr in range(4):
                    nc.tensor.matmul(
                        ps[:, sl],
                        xc[:, 2 * pr : 2 * pr + 2, jp : jp + 128],
                        wu_s[:, 2 * pr : 2 * pr + 2, :],
                        start=False, stop=(last and pr == 3), perf_mode=DRm,
                    )
                if j % 4 == 3:
                    nc.vector.tensor_copy(uT[:, b * 4 : b * 4 + 4, :], u_ps.pop(b))

            # ---------------- per-slot conv work lists ----------------
            # work[s] runs after S/O in slot s; pre[s] runs before the S
            # emission (for deps feeding that S). Only 2 conv PSUM bufs exist,
            # so at most a k-stream tile and a u-batch are live at once in
            # ib0; q tiles 0..2 build in the prologue, tile 3 in the ib0 tail
            # after the k/u streams drain, tiles 4+ mid-ib (nothing else live).
            work = [[] for _ in range(NSLOT)]
            pre = [[] for _ in range(NSLOT)]
            prologue = []

            # prologue: k0, q0, q1 (what S(0) needs) and u jts 0..3 (the
            # per-jt O matmul consumes uT from slot 0).
            for g in range(4):
                prologue.append(lambda g=g: qk_group(wk_s, bk_s, kb, 0, g))
            for t in range(2):
                for g in range(4):
                    prologue.append(lambda t=t, g=g: qk_group(wq_s, bq_s, qb, t, g))
            for j in range(4):
                prologue.append(lambda j=j: u_jt(j))
            # u jts 4..31, monotone schedule with >=3 slots of lead;
            # interleaved k/u windows keep the 2 conv-PSUM buffers alternating.
            us = 0
            for j in range(4, NJT):
                us = max(us, 1, j - 6)
                work[us].append(lambda j=j: u_jt(j))
            # k tiles 1..7: 4 groups at slots 4T-6..4T-3 (clamped)
            for T in range(1, 8):
                for g in range(4):
                    work[max(0, 4 * T - 6 + g)].append(
                        lambda T=T, g=g: qk_group(wk_s, bk_s, kb, T, g)
                    )
            # q tiles 2,3 in the ib0 tail (after the k/u streams drain), with
            # the q3 eviction landing a full slot before the S(32) emission.
            for gg, s in zip(range(4), (26, 26, 27, 27)):
                work[s].append(lambda g=gg: qk_group(wq_s, bq_s, qb, 2, g))
            for gg, s in zip(range(4), (28, 28, 29, 29)):
                work[s].append(lambda g=gg: qk_group(wq_s, bq_s, qb, 3, g))
            # q tiles for ib i+1 (i>=1): 8 groups in ib i slots 16..23
            for i in range(1, NIB - 1):
                for gg in range(8):
                    t, g = 2 * i + 2 + gg // 4, gg % 4
                    work[32 * i + 16 + gg].append(
                        lambda t=t, g=g: qk_group(wq_s, bq_s, qb, t, g)
                    )

            # ---------------- S matmul ----------------
            def s_mm(gs):
                ib, jt = gs // NJT, gs % NJT
                sp = sps.tile([C, IB], F32, tag="sp", name="sp")
                for h in range(IB // 512):
                    nc.tensor.matmul(
                        sp[:, h * 512 : (h + 1) * 512],
                        kb[:, jt * 128 : (jt + 1) * 128],
                        qb[:, ib * IB + h * 512 : ib * IB + (h + 1) * 512],
                        start=True, stop=True,
                    )
                return sp

            # ---------------- main loop ----------------
            for fn in prologue:
                fn()

            sp = s_mm(0)
            acc_pending = None
            for ib in range(NIB):
                isl = slice(ib * IB, (ib + 1) * IB)
                ob = ops.tile([C, IB], F32, tag="ob", name="ob")
                accs = ap.tile([C, IB], F16, tag="accs", name="accs")
                for jt in range(NJT):
                    gs = ib * NJT + jt
                    ae = AE if (ib >= 1 and ED > 0) else IB
                    e2 = ep.tile([C, IB], F16, tag="e", name="e")
                    nc.scalar.activation(
                        e2[:, 0:ae], sp[:, 0:ae],
                        mybir.ActivationFunctionType.Exp,
                        bias=ebias, scale=SCALE / (WSCALE * WSCALE),
                    )
                    if ae < IB:
                        # fp16 2^x bits built by the DVE straight into e2
                        nc.vector.tensor_scalar(
                            e2[:, ae:IB].bitcast(mybir.dt.int16), sp[:, ae:IB],
                            SCALE / (WSCALE * WSCALE) * A16,
                            EXP_BIAS * A16 + K0,
                            mybir.AluOpType.mult, mybir.AluOpType.add,
                        )
                    for fn in pre[gs]:
                        fn()
                    if gs + 1 < NSLOT:
                        sp = s_mm(gs + 1)
                    for h in range(2):
                        nc.tensor.matmul(
                            ob[:, h * 512 : (h + 1) * 512],
                            uT[:, jt, :],
                            e2[:, h * 512 : (h + 1) * 512],
                            start=(jt == 0), stop=(jt == NJT - 1),
                        )
                    for fn in work[gs]:
                        fn()

                    # colsum partial accumulation (fp16, low 3/4 on DVE and
                    # high 1/4 on the otherwise-idle Pool engine), run one jt
                    # behind so each slot's DVE starts with the exp trick —
                    # the trick holds the S PSUM buffer, so running it first
                    # unblocks the next-but-one S matmul early.
                    def acc_fn(accs=accs, jt=jt, e2=e2):
                        if jt == 0:
                            nc.vector.tensor_copy(accs[:, 0:768], e2[:, 0:768])
                            nc.gpsimd.tensor_copy(accs[:, 768:IB], e2[:, 768:IB])
                        else:
                            nc.vector.tensor_add(
                                accs[:, 0:768], accs[:, 0:768], e2[:, 0:768]
                            )
                            nc.gpsimd.tensor_tensor(
                                accs[:, 768:IB], accs[:, 768:IB], e2[:, 768:IB],
                                mybir.AluOpType.add,
                            )

                    if acc_pending is not None:
                        acc_pending()
                    acc_pending = acc_fn

                # ---- per-ib tail: colsum reduce, reciprocal, normalize ----
                acc_pending()
                acc_pending = None
                if ib < NIB - 1:
                    # mid-run: DRAM-bounce reciprocal broadcast; the DMA
                    # latency and the normalize chunks hide under the next
                    # ib's compute (chunks deferred into its work slots so
                    # the DVE isn't swamped right at the boundary).
                    obe = fin.tile([C, IB], F32, tag="obe", name="obe")
                    nc.vector.tensor_copy(obe, ob)
                    cst = cps.tile([C, 8], F32, tag="conv", name="cst")
                    accs_v = accs.rearrange("p (a b) -> p a b", b=8)
                    for c8 in range(8):
                        nc.tensor.matmul(
                            cst[:, c8 : c8 + 1], accs_v[:, :, c8], ones,
                            start=True, stop=True,
                        )
                    nc.vector.reciprocal(rcp[:, ib, :], cst)
                    nc.sync.dma_start(rstage[isl], rcp[:, ib, :])
                    rb = fin.tile([C, IB], F32, tag="rb", name="rb")
                    nc.sync.dma_start(rb, rstage[isl].partition_broadcast(C))

                    def norm_chunk(chk, ib=ib, obe=obe, rb=rb):
                        csl = slice(ib * IB + chk * 256, ib * IB + (chk + 1) * 256)
                        nt = fin.tile([C, 256], F32, tag="nt", name="nt")
                        nc.vector.tensor_mul(
                            nt, obe[:, chk * 256 : (chk + 1) * 256],
                            rb[:, chk * 256 : (chk + 1) * 256],
                        )
                        ot = fin.tile([C, 256], F16, tag="ot", name="ot")
                        nc.vector.tensor_add(ot, nt, xres[:, csl])
                        nc.sync.dma_start(
                            out_d[:, ib * 16 + chk * 4 : ib * 16 + (chk + 1) * 4, :],
                            ot,
                        )

                    for chk in range(4):
                        work[(ib + 1) * 32 + 4 + 2 * chk].append(
                            lambda chk=chk: norm_chunk(chk)
                        )
                else:
                    # final ib: same DRAM-bounce broadcast (the [128,8]
                    # reciprocal is ~free; a free-dim-wide reciprocal costs
                    # microseconds on real DVE), finely chunked, residual
                    # adds alternating DVE/Pool, stores alternating queues.
                    cst = cps.tile([C, 8], F32, tag="conv", name="cst")
                    accs_v = accs.rearrange("p (a b) -> p a b", b=8)
                    for c8 in range(8):
                        nc.tensor.matmul(
                            cst[:, c8 : c8 + 1], accs_v[:, :, c8], ones,
                            start=True, stop=True,
                        )
                    nc.vector.reciprocal(rcp[:, ib, :], cst)
                    nc.sync.dma_start(rstage[isl], rcp[:, ib, :])
                    rb = fin.tile([C, IB], F32, tag="rb", name="rb")
                    nc.sync.dma_start(rb, rstage[isl].partition_broadcast(C))
                    for chk in range(4):
                        csl = slice(ib * IB + chk * 256, ib * IB + (chk + 1) * 256)
                        nt = fin.tile([C, 256], F32, tag="nt", name="nt")
                        nc.vector.tensor_mul(
                            nt, ob[:, chk * 256 : (chk + 1) * 256],
                            rb[:, chk * 256 : (chk + 1) * 256],
                        )
                        ot = fin.tile([C, 256], F16, tag="ot", name="ot")
                        eng = nc.vector if chk % 2 == 0 else nc.gpsimd
                        eng.tensor_tensor(
                            ot, nt, xres[:, csl], mybir.AluOpType.add
                        )
                        qeng = nc.sync if chk % 2 == 0 else nc.scalar
                        qeng.dma_start(
                            out_d[:, ib * 16 + chk * 4 : ib * 16 + (chk + 1) * 4, :],
                            ot,
                        )
            dsp.release()
            fin.release()
            ap.release()
            ep.release()
            ops.release()
            sps.release()
            cps.release()

    nc.finalize()
    return nc


def get_nc():
    if "nc" not in _CACHE:
        _CACHE["nc"] = _build_nc()
    return _CACHE["nc"]


def _prep_host_inputs(x, Wq, bq, Wk, bk, Wv, bv, Wo, bo):
    x = np.ascontiguousarray(np.asarray(x, dtype=np.float32))
    Wq = np.asarray(Wq, dtype=np.float32)
    Wk = np.asarray(Wk, dtype=np.float32)
    Wv = np.asarray(Wv, dtype=np.float64)
    Wo2 = np.asarray(Wo, dtype=np.float64).reshape(C, C)
    bq = np.asarray(bq, dtype=np.float32)
    bk = np.asarray(bk, dtype=np.float32)
    bv = np.asarray(bv, dtype=np.float64)
    bo = np.asarray(bo, dtype=np.float64)

    # lhsT layouts: w[c, tap, o] = W[o, c, dy, dx], pre-scaled into fp8 range
    wq = np.ascontiguousarray(Wq.transpose(1, 2, 3, 0).reshape(C, NTAP, C)) * WSCALE
    wk = np.ascontiguousarray(Wk.transpose(1, 2, 3, 0).reshape(C, NTAP, C)) * WSCALE
    Wu = np.einsum("om,mckl->ockl", Wo2, Wv)
    wu = np.ascontiguousarray(Wu.transpose(1, 2, 3, 0).reshape(C, NTAP, C)) * WSCALE
    bu = (Wo2 @ bv + bo).astype(np.float32)

    # img2col in fp8: xcol[b, c, tap, pix] = xpad[b, c, py+dy, px+dx]
    xpad = np.pad(x, ((0, 0), (0, 0), (1, 1), (1, 1)))
    s0, s1, s2, s3 = xpad.strides
    win = np.lib.stride_tricks.as_strided(
        xpad, shape=(B, C, 3, 3, H, W), strides=(s0, s1, s2, s3, s2, s3)
    )
    xcol = np.ascontiguousarray(
        win.transpose(0, 1, 2, 3, 4, 5).reshape(B, C, NTAP, N)
    ).astype(NP8)

    xres = (x.reshape(B, C, N) + bu[None, :, None]).astype(np.float16)

    wks = np.ascontiguousarray(
        np.stack([wk, wq, wu], axis=1)
    ).astype(NP8)  # [C, 3(k,q,u), NTAP, C]
    bqk = np.ascontiguousarray(
        np.stack([bq * WSCALE, bk * WSCALE], axis=1)
    ).astype(np.float32)
    shared = {"wks": wks, "bqk": bqk}
    in_maps = [
        dict(shared, xc=np.ascontiguousarray(xcol[i]), xr=np.ascontiguousarray(xres[i]))
        for i in range(B)
    ]
    return in_maps


def _run(inputs, trace=False):
    from concourse.bass_utils import run_bass_kernel_spmd

    in_maps = _prep_host_inputs(**inputs)
    nc = get_nc()
    res = run_bass_kernel_spmd(nc, in_maps, core_ids=list(range(B)), trace=trace)
    out = np.stack([np.asarray(res.results[i]["out"]) for i in range(B)])
    return out.reshape(B, C, H, W).astype(np.float32), res


def kernel(**inputs) -> np.ndarray:
    out, _ = _run(inputs, trace=False)
    return out


# revision 11
# speedup vs baseline: 1.0678x; 1.0678x over previous
# Trainium2 Bass kernel for AttentionBlock (conv-qkv + spatial softmax attention
# + 1x1 conv out + residual), data-parallel over batch on 8 NeuronCores.
#
# v2: engine-balanced rewrite of the fp16 baseline.
#   - All convs run from an img2col fp8 image (xc [c, 9tap, 4096pix], built on
#     host) as DoubleRow fp8 matmuls (2 taps contracted per pass, 0.5 cyc/row).
#   - The u-conv (Wo.Wv folded) is computed directly TRANSPOSED on the PE
#     (stationary = image window, moving = weights) so uT needs no transposes.
#   - Attention: S = k^T q stays fp16 (PSUM-write-bound either way); exp runs
#     on ACT with fp8e4 output; O = uT^T E runs as DoubleRow fp8 over jt-pairs.
#   - colsum (softmax denominator) accumulates in fp16 via DVE+Pool split.
#   - Conv work is streamed into the attention loop (ACT is the critical
#     engine; PE slack absorbs the convs), with explicit per-slot work lists.
#   - Weights are scaled x32 on host to dodge fp8 subnormals; the 1/32 (and
#     the 1/1024 logit scale) fold into the activation scale and the final
#     normalize (scalar_tensor_tensor), costing nothing.
#   - bias bu = Wo bv + bo commutes with the row-stochastic A and is folded
#     into the residual (host adds it to xres). bk cancels in softmax but is
#     applied anyway; bq is applied on q eviction.

import numpy as np

try:
    import concourse.bass as bass  # noqa: F401
except ImportError:  # pragma: no cover
    import sys

    sys.path.insert(0, "/opt/trn_rl_repo")

import concourse.bass as bass  # noqa: F401
import concourse.mybir as mybir
import ml_dtypes
from concourse import bacc
from concourse import tile

B = 8
C = 128
H = W = 64
N = H * W  # 4096
NTAP = 9
IB = 1024  # attention i-block (columns per PSUM residency)
NIB = N // IB  # 4
NJT = N // 128  # 32 j-tiles per ib
NSLOT = NIB * NJT  # 128 global slots
WSCALE = 32.0  # fp8 weight pre-scale (host)
SCALE = float(C) ** -0.5
EXP_BIAS = -3.0

F32 = mybir.dt.float32
F16 = mybir.dt.float16
F8 = mybir.dt.float8e4
NP8 = ml_dtypes.float8_e4m3
DRm = mybir.MatmulPerfMode.DoubleRow

# DVE-exp share (ib>=1): the last ED columns of each S tile are exponentiated
# on the DVE via the int16 Schraudolph bit-trick (bits of fp16 2^x built by a
# single mult+add+convert), written straight into e2's fp16 tail columns so
# the O matmul and the colsum read one uniform fp16 tile. The 2^K0 offset the
# trick introduces is uniform per column and cancels in the per-column
# softmax normalization. K0 is chosen so the int16 can never go negative
# (negative bit patterns would alias to fp16 -NaN).
ED = 128  # 0 disables the DVE-exp path
AE = IB - ED  # ACT-exp columns per tile in ib>=1
A16 = 1024.0 * 1.4426950408889634
K0 = 17156.0

_CACHE = {}


def _build_nc():
    nc = bacc.Bacc(None)

    xc_d = nc.dram_tensor("xc", [C, NTAP, N], F8, kind="ExternalInput")
    # weights and biases packed into single tensors: DMA *issue* time on the
    # sequencer (~0.6-1.3us each) dominates the prologue, so fewer DMAs win.
    wks_d = nc.dram_tensor("wks", [C, 3, NTAP, C], F8, kind="ExternalInput")
    bqk_d = nc.dram_tensor("bqk", [C, 2], F32, kind="ExternalInput")
    xr_d = nc.dram_tensor("xr", [C, N], F16, kind="ExternalInput")
    out_d = nc.dram_tensor("out", [C, H, W], F16, kind="ExternalOutput")

    with tile.TileContext(nc) as tc:
        with tc.tile_pool(name="persist", bufs=1) as pp:
            xc = pp.tile([C, NTAP, N], F8)
            xres = pp.tile([C, N], F16)
            qb = pp.tile([C, N], F16)
            kb = pp.tile([C, N], F16)
            uT = pp.tile([C, NJT, C], F16)  # [j-in-tile, jt, c]
            wks = pp.tile([C, 3, NTAP, C], F8)  # packed wk, wq, wu
            bqk = pp.tile([C, 2], F32)
            wk_s, wq_s, wu_s = wks[:, 0], wks[:, 1], wks[:, 2]
            bq_s, bk_s = bqk[:, 0:1], bqk[:, 1:2]
            ebias = pp.tile([C, 1], F32)
            ones = pp.tile([C, 1], F16)
            rcp = pp.tile([C, NIB, 8], F32)

            # -------- input DMAs, spread over 4 queues so the early chunks
            # land in parallel (per-queue issue+transfer is serial) --------
            # One queue, strictly in consumption order: both the issue slots
            # and the transfer channel are serial, so order is everything and
            # fewer DMAs are better.
            nc.sync.dma_start(wks, wks_d[:])
            nc.sync.dma_start(bqk, bqk_d[:])
            for ch in range(4):
                nc.sync.dma_start(
                    xc[:, :, ch * 1024 : (ch + 1) * 1024],
                    xc_d[:, :, ch * 1024 : (ch + 1) * 1024],
                )
            nc.sync.dma_start(xres, xr_d[:])
            # PE warmup: the tensor engine ramps to full clock only after
            # ~3us from its first instruction; issue throwaway matmuls on
            # memset data immediately so the ramp overlaps the input DMAs.
            wrm = pp.tile([C, 512], F16)
            nc.vector.memset(wrm, 0.0)
            nc.vector.memset(ebias, EXP_BIAS)
            # ones = WSCALE: folds the 1/WSCALE u-descale into the softmax
            # denominator (colsum*WSCALE), so normalize is a plain multiply.
            nc.vector.memset(ones, WSCALE)

            cps = tc.alloc_tile_pool(name="cps", bufs=2, space="PSUM")
            # continuous warmup until the first conv's inputs arrive — if the
            # PE goes idle the clock governor drops back to the mid p-state.
            wps = cps.tile([C, 512], F32, tag="conv", name="wps")
            for _ in range(8):
                nc.tensor.matmul(
                    wps[0:64, :], wrm[:, 0:64], wrm, start=True, stop=True
                )
            sps = tc.alloc_tile_pool(name="sps", bufs=2, space="PSUM")
            ops = tc.alloc_tile_pool(name="ops", bufs=1, space="PSUM")
            ep = tc.alloc_tile_pool(name="ep", bufs=4)
            ap = tc.alloc_tile_pool(name="accp", bufs=2)
            fin = tc.alloc_tile_pool(name="fin", bufs=2)
            dsp = tc.alloc_tile_pool(name="dstage", bufs=1, space="DRAM")
            rstage = dsp.tile([N], F32)

            # ---------------- conv emission helpers ----------------
            # q/k conv tile t covers pixels [t*512, (t+1)*512), normal
            # orientation (out channels on partitions). Emitted as 4 groups:
            # group 0 = tap 8 (plain fp8, the sim's group anchor) + pair 0,
            # groups 1,2 = pairs 1,2, group 3 = pair 3 + eviction (DVE).
            # The eviction is a full slot ahead of the next tile's group 0,
            # so its DVE round-trip hides under the S/O/u work in between.
            qk_ps = {}

            def qk_pair(ps, w_s, p0, pr):
                for ph in range(2):
                    nc.tensor.matmul(
                        ps[:, ph * 256 : (ph + 1) * 256],
                        w_s[:, 2 * pr : 2 * pr + 2, :],
                        xc[:, 2 * pr : 2 * pr + 2, p0 + ph * 256 : p0 + (ph + 1) * 256],
                        start=False, stop=(pr == 3 and ph == 1), perf_mode=DRm,
                    )

            def qk_group(w_s, b_s, dest, t, g):
                p0 = t * 512
                if g == 0:
                    ps = cps.tile([C, 512], F32, tag="conv", name="cps")
                    qk_ps[id(w_s), t] = ps
                    nc.tensor.matmul(
                        ps, w_s[:, 8, :], xc[:, 8, p0 : p0 + 512],
                        start=True, stop=False,
                    )
                    qk_pair(ps, w_s, p0, 0)
                    return
                ps = qk_ps[id(w_s), t]
                qk_pair(ps, w_s, p0, g)
                if g == 3:
                    del qk_ps[id(w_s), t]
                    nc.vector.tensor_scalar_add(
                        dest[:, p0 : p0 + 512], ps, b_s
                    )

            # u-conv j-tile j (transposed): stationary = image window,
            # moving = wu. Lands [128 pix, 128 ch] into a 4-jt psu batch.
            u_ps = {}

            def u_jt(j):
                b = j // 4
                if j % 4 == 0:
                    u_ps[b] = cps.tile([C, 512], F32, tag="conv", name="ups")
                ps = u_ps[b]
                sl = slice((j % 4) * 128, (j % 4 + 1) * 128)
                jp = j * 128
                first = j % 4 == 0
                last = j % 4 == 3
                nc.tensor.matmul(
                    ps[:, sl], xc[:, 8, jp : jp + 128], wu_s[:, 8, :],
                    start=first, stop=False,
                )
                for pr in range(4):
                    nc.tensor.matmul(
                        ps[:, sl],
                        xc[:, 2 * pr : 2 * pr + 2, jp : jp + 128],
                        wu_s[:, 2 * pr : 2 * pr + 2, :],
                        start=False, stop=(last and pr == 3), perf_mode=DRm,
                    )
                if j % 4 == 3:
                    nc.vector.tensor_copy(uT[:, b * 4 : b * 4 + 4, :], u_ps.pop(b))

            # ---------------- per-slot conv work lists ----------------
            # work[s] runs after S/O in slot s; pre[s] runs before the S
            # emission (for deps feeding that S). Only 2 conv PSUM bufs exist,
            # so at most a k-stream tile and a u-batch are live at once in
            # ib0; q tiles 0..2 build in the prologue, tile 3 in the ib0 tail
            # after the k/u streams drain, tiles 4+ mid-ib (nothing else live).
            work = [[] for _ in range(NSLOT)]
            pre = [[] for _ in range(NSLOT)]
            prologue = []

            # prologue: k0, q0, q1 (what S(0) needs) and u jts 0..3 (the
            # per-jt O matmul consumes uT from slot 0).
            for g in range(4):
                prologue.append(lambda g=g: qk_group(wk_s, bk_s, kb, 0, g))
            for t in range(2):
                for g in range(4):
                    prologue.append(lambda t=t, g=g: qk_group(wq_s, bq_s, qb, t, g))
            for j in range(4):
                prologue.append(lambda j=j: u_jt(j))
            # u jts 4..31, monotone schedule with >=3 slots of lead;
            # interleaved k/u windows keep the 2 conv-PSUM buffers alternating.
            us = 0
            for j in range(4, NJT):
                us = max(us, 1, j - 6)
                work[us].append(lambda j=j: u_jt(j))
            # k tiles 1..7: 4 groups at slots 4T-6..4T-3 (clamped)
            for T in range(1, 8):
                for g in range(4):
                    work[max(0, 4 * T - 6 + g)].append(
                        lambda T=T, g=g: qk_group(wk_s, bk_s, kb, T, g)
                    )
            # q tiles 2,3 in the ib0 tail (after the k/u streams drain), with
            # the q3 eviction landing a full slot before the S(32) emission.
            for gg, s in zip(range(4), (26, 26, 27, 27)):
                work[s].append(lambda g=gg: qk_group(wq_s, bq_s, qb, 2, g))
            for gg, s in zip(range(4), (28, 28, 29, 29)):
                work[s].append(lambda g=gg: qk_group(wq_s, bq_s, qb, 3, g))
            # q tiles for ib i+1 (i>=1): 8 groups in ib i slots 16..23
            for i in range(1, NIB - 1):
                for gg in range(8):
                    t, g = 2 * i + 2 + gg // 4, gg % 4
                    work[32 * i + 16 + gg].append(
                        lambda t=t, g=g: qk_group(wq_s, bq_s, qb, t, g)
                    )

            # ---------------- S matmul ----------------
            def s_mm(gs):
                ib, jt = gs // NJT, gs % NJT
                sp = sps.tile([C, IB], F32, tag="sp", name="sp")
                for h in range(IB // 512):
                    nc.tensor.matmul(
                        sp[:, h * 512 : (h + 1) * 512],
                        kb[:, jt * 128 : (jt + 1) * 128],
                        qb[:, ib * IB + h * 512 : ib * IB + (h + 1) * 512],
                        start=True, stop=True,
                    )
                return sp

            # ---------------- main loop ----------------
            for fn in prologue:
                fn()

            sp = s_mm(0)
            acc_pending = None
            for ib in range(NIB):
                isl = slice(ib * IB, (ib + 1) * IB)
                ob = ops.tile([C, IB], F32, tag="ob", name="ob")
                accs = ap.tile([C, IB], F16, tag="accs", name="accs")
                for jt in range(NJT):
                    gs = ib * NJT + jt
                    ae = AE if (ib >= 1 and ED > 0) else IB
                    e2 = ep.tile([C, IB], F16, tag="e", name="e")
                    nc.scalar.activation(
                        e2[:, 0:ae], sp[:, 0:ae],
                        mybir.ActivationFunctionType.Exp,
                        bias=ebias, scale=SCALE / (WSCALE * WSCALE),
                    )
                    if ae < IB:
                        # fp16 2^x bits built by the DVE straight into e2
                        nc.vector.tensor_scalar(
                            e2[:, ae:IB].bitcast(mybir.dt.int16), sp[:, ae:IB],
                            SCALE / (WSCALE * WSCALE) * A16,
                            EXP_BIAS * A16 + K0,
                            mybir.AluOpType.mult, mybir.AluOpType.add,
                        )
                    for fn in pre[gs]:
                        fn()
                    if gs + 1 < NSLOT:
                        sp = s_mm(gs + 1)
                    for h in range(2):
                        nc.tensor.matmul(
                            ob[:, h * 512 : (h + 1) * 512],
                            uT[:, jt, :],
                            e2[:, h * 512 : (h + 1) * 512],
                            start=(jt == 0), stop=(jt == NJT - 1),
                        )
                    for fn in work[gs]:
                        fn()

                    # colsum partial accumulation (single fp16 DVE op), run
                    # one jt behind so each slot's DVE starts with the exp
                    # trick — the trick holds the S PSUM buffer, so running
                    # it first unblocks the next-but-one S matmul early.
                    def acc_fn(accs=accs, jt=jt, e2=e2):
                        if jt == 0:
                            nc.vector.tensor_copy(accs, e2)
                        else:
                            nc.vector.tensor_add(accs, accs, e2)

                    if acc_pending is not None:
                        acc_pending()
                    acc_pending = acc_fn

                # ---- per-ib tail: colsum reduce, reciprocal, normalize ----
                acc_pending()
                acc_pending = None
                if ib < NIB - 1:
                    # mid-run: DRAM-bounce reciprocal broadcast; the DMA
                    # latency and the normalize chunks hide under the next
                    # ib's compute (chunks deferred into its work slots so
                    # the DVE isn't swamped right at the boundary).
                    obe = fin.tile([C, IB], F32, tag="obe", name="obe")
                    nc.vector.tensor_copy(obe, ob)
                    cst = cps.tile([C, 8], F32, tag="conv", name="cst")
                    accs_v = accs.rearrange("p (a b) -> p a b", b=8)
                    for c8 in range(8):
                        nc.tensor.matmul(
                            cst[:, c8 : c8 + 1], accs_v[:, :, c8], ones,
                            start=True, stop=True,
                        )
                    nc.vector.reciprocal(rcp[:, ib, :], cst)
                    nc.sync.dma_start(rstage[isl], rcp[:, ib, :])
                    rb = fin.tile([C, IB], F32, tag="rb", name="rb")
                    nc.sync.dma_start(rb, rstage[isl].partition_broadcast(C))

                    def norm_chunk(chk, ib=ib, obe=obe, rb=rb):
                        csl = slice(ib * IB + chk * 256, ib * IB + (chk + 1) * 256)
                        nt = fin.tile([C, 256], F32, tag="nt", name="nt")
                        nc.vector.tensor_mul(
                            nt, obe[:, chk * 256 : (chk + 1) * 256],
                            rb[:, chk * 256 : (chk + 1) * 256],
                        )
                        ot = fin.tile([C, 256], F16, tag="ot", name="ot")
                        nc.vector.tensor_add(ot, nt, xres[:, csl])
                        nc.sync.dma_start(
                            out_d[:, ib * 16 + chk * 4 : ib * 16 + (chk + 1) * 4, :],
                            ot,
                        )

                    for chk in range(4):
                        work[(ib + 1) * 32 + 4 + 2 * chk].append(
                            lambda chk=chk: norm_chunk(chk)
                        )
                else:
                    # final ib: same DRAM-bounce broadcast (the [128,8]
                    # reciprocal is ~free; a free-dim-wide reciprocal costs
                    # microseconds on real DVE), finely chunked, residual
                    # adds alternating DVE/Pool, stores alternating queues.
                    cst = cps.tile([C, 8], F32, tag="conv", name="cst")
                    accs_v = accs.rearrange("p (a b) -> p a b", b=8)
                    for c8 in range(8):
                        nc.tensor.matmul(
                            cst[:, c8 : c8 + 1], accs_v[:, :, c8], ones,
                            start=True, stop=True,
                        )
                    nc.vector.reciprocal(rcp[:, ib, :], cst)
                    nc.sync.dma_start(rstage[isl], rcp[:, ib, :])
                    rb = fin.tile([C, IB], F32, tag="rb", name="rb")
                    nc.sync.dma_start(rb, rstage[isl].partition_broadcast(C))
                    for chk in range(4):
                        csl = slice(ib * IB + chk * 256, ib * IB + (chk + 1) * 256)
                        nt = fin.tile([C, 256], F32, tag="nt", name="nt")
                        nc.vector.tensor_mul(
                            nt, ob[:, chk * 256 : (chk + 1) * 256],
                            rb[:, chk * 256 : (chk + 1) * 256],
                        )
                        ot = fin.tile([C, 256], F16, tag="ot", name="ot")
                        eng = nc.vector if chk % 2 == 0 else nc.gpsimd
                        eng.tensor_tensor(
                            ot, nt, xres[:, csl], mybir.AluOpType.add
                        )
                        qeng = nc.sync if chk % 2 == 0 else nc.scalar
                        qeng.dma_start(
                            out_d[:, ib * 16 + chk * 4 : ib * 16 + (chk + 1) * 4, :],
                            ot,
                        )
            dsp.release()
            fin.release()
            ap.release()
            ep.release()
            ops.release()
            sps.release()
            cps.release()

    nc.finalize()
    return nc


def get_nc():
    if "nc" not in _CACHE:
        _CACHE["nc"] = _build_nc()
    return _CACHE["nc"]


def _prep_host_inputs(x, Wq, bq, Wk, bk, Wv, bv, Wo, bo):
    x = np.ascontiguousarray(np.asarray(x, dtype=np.float32))
    Wq = np.asarray(Wq, dtype=np.float32)
    Wk = np.asarray(Wk, dtype=np.float32)
    Wv = np.asarray(Wv, dtype=np.float64)
    Wo2 = np.asarray(Wo, dtype=np.float64).reshape(C, C)
    bq = np.asarray(bq, dtype=np.float32)
    bk = np.asarray(bk, dtype=np.float32)
    bv = np.asarray(bv, dtype=np.float64)
    bo = np.asarray(bo, dtype=np.float64)

    # lhsT layouts: w[c, tap, o] = W[o, c, dy, dx], pre-scaled into fp8 range
    wq = np.ascontiguousarray(Wq.transpose(1, 2, 3, 0).reshape(C, NTAP, C)) * WSCALE
    wk = np.ascontiguousarray(Wk.transpose(1, 2, 3, 0).reshape(C, NTAP, C)) * WSCALE
    Wu = np.einsum("om,mckl->ockl", Wo2, Wv)
    wu = np.ascontiguousarray(Wu.transpose(1, 2, 3, 0).reshape(C, NTAP, C)) * WSCALE
    bu = (Wo2 @ bv + bo).astype(np.float32)

    # img2col in fp8: xcol[b, c, tap, pix] = xpad[b, c, py+dy, px+dx]
    xpad = np.pad(x, ((0, 0), (0, 0), (1, 1), (1, 1)))
    s0, s1, s2, s3 = xpad.strides
    win = np.lib.stride_tricks.as_strided(
        xpad, shape=(B, C, 3, 3, H, W), strides=(s0, s1, s2, s3, s2, s3)
    )
    xcol = np.ascontiguousarray(
        win.transpose(0, 1, 2, 3, 4, 5).reshape(B, C, NTAP, N)
    ).astype(NP8)

    xres = (x.reshape(B, C, N) + bu[None, :, None]).astype(np.float16)

    wks = np.ascontiguousarray(
        np.stack([wk, wq, wu], axis=1)
    ).astype(NP8)  # [C, 3(k,q,u), NTAP, C]
    bqk = np.ascontiguousarray(
        np.stack([bq * WSCALE, bk * WSCALE], axis=1)
    ).astype(np.float32)
    shared = {"wks": wks, "bqk": bqk}
    in_maps = [
        dict(shared, xc=np.ascontiguousarray(xcol[i]), xr=np.ascontiguousarray(xres[i]))
        for i in range(B)
    ]
    return in_maps


def _run(inputs, trace=False):
    from concourse.bass_utils import run_bass_kernel_spmd

    in_maps = _prep_host_inputs(**inputs)
    nc = get_nc()
    res = run_bass_kernel_spmd(nc, in_maps, core_ids=list(range(B)), trace=trace)
    out = np.stack([np.asarray(res.results[i]["out"]) for i in range(B)])
    return out.reshape(B, C, H, W).astype(np.float32), res


def kernel(**inputs) -> np.ndarray:
    out, _ = _run(inputs, trace=False)
    return out
